# revision 18
# baseline (speedup 1.0000x reference)
"""GCN encoder (2x GCNConv + ReLU + global mean pool) on 8 Trainium2 cores.

Sharding: nodes (and their incident in-edges) are partitioned across the 8
cores; GCN weights are replicated; per-graph pooled sums are computed as
per-core partials and combined on the host during unsharding.

Self-contained: hardcodes the problem shapes from the task spec.
"""
import sys
sys.path.insert(0, '/opt/trn_rl_repo')

import numpy as np
import ml_dtypes

BF16NP = np.dtype(ml_dtypes.bfloat16)

# Problem shapes (fixed by the task).
N, E, IN_C, HID, OUT_C = 50000, 800000, 768, 256, 128
NCORES = 8
NPC = N // NCORES          # 6250 nodes owned per core
P = 128
TPT = 49                   # dst tiles per core (49*128 = 6272)
NPCP = TPT * P             # padded nodes per core
WG = 64                    # per-core graph window for pooling
KCH1 = IN_C // P           # 6 k-chunks for x @ W1


def _split_multi_waits(nc, mybir):
    """This walrus build rejects >1 sync-wait per instruction; move extra
    waits onto single-wait NoOp carriers inserted just before, same engine."""
    for fn in nc.m.functions:
        for blk in fn.blocks:
            insts = blk.instructions
            pos = 0
            while pos < len(insts):
                inst = insts[pos]
                si = inst.sync_info
                if si is not None and len(si.on_wait) > 1:
                    w = list(si.on_wait)
                    u = list(si.on_update)
                    newds = []
                    for j, wj in enumerate(w[:-1]):
                        d = mybir.InstNoOp(name=f"{inst.name}-sp{j}", ins=[], outs=[])
                        d.engine = inst.engine
                        d.sync_info = mybir.SyncInfo(on_wait=[wj], on_update=[])
                        newds.append(d)
                    inst.sync_info = mybir.SyncInfo(on_wait=[w[-1]], on_update=u)
                    insts[pos:pos] = newds
                    pos += len(newds)
                pos += 1


def _preprocess(x, edge_index, batch, W1, b1, W2, b2):
    """Host-side sharding / index preprocessing. Returns (in_maps, meta)."""
    x = np.asarray(x, np.float32)
    src = np.asarray(edge_index[0], np.int64)
    dst = np.asarray(edge_index[1], np.int64)
    batch = np.asarray(batch, np.int64)

    deg = np.bincount(dst, minlength=N).astype(np.float32) + 1.0
    dinv = 1.0 / np.sqrt(deg)

    # Global node id -> padded table row. The tables are assembled from
    # chunked AllGathers (rank-major within each chunk), so the row layout
    # depends on the chunk boundaries (in local padded rows).
    def make_table_idx(bounds):
        def table_idx(u):
            r, l = u // NPC, u % NPC
            row = np.zeros_like(u)
            base = 0
            for lo, hi in zip(bounds[:-1], bounds[1:]):
                sel = (l >= lo) & (l < hi)
                row[sel] = base + r[sel] * (hi - lo) + (l[sel] - lo)
                base += NCORES * (hi - lo)
            return row
        return table_idx
    S_BOUNDS = [0, NPCP]                # AG1 unsplit (sim: fixed cost per collective dominates)
    Z_BOUNDS = [0, 3584, NPCP]          # AG2 halves (z-groups 0..6 / 7..12)
    s_table_idx = make_table_idx(S_BOUNDS)
    z_table_idx = make_table_idx(Z_BOUNDS)

    owner = dst // NPC
    tile_of = (dst % NPC) // P
    is_rem = np.ones(E, bool)   # single padded run per tile (split costs more than it hides)

    # per-(core, tile) local/remote counts -> shared padded schedule
    cnt_loc = np.zeros((NCORES, TPT), np.int64)
    cnt_rem = np.zeros((NCORES, TPT), np.int64)
    np.add.at(cnt_loc, (owner[~is_rem], tile_of[~is_rem]), 1)
    np.add.at(cnt_rem, (owner[is_rem], tile_of[is_rem]), 1)
    K_loc = -(-cnt_loc.max(axis=0) // P)
    K_rem = np.maximum(1, -(-cnt_rem.max(axis=0) // P))
    K = K_loc + K_rem
    off = np.concatenate([[0], np.cumsum(K)])[:-1]
    T_pad = int(K.sum())

    # order edges by (owner, tile, remote?, dst_local) and fill padded slots
    order = np.lexsort((dst, is_rem, tile_of, owner))
    so, do_, oo, to, ro = src[order], dst[order], owner[order], tile_of[order], is_rem[order]

    srcidx_all = np.zeros((NCORES, T_pad * P), np.int32)
    srcidx2_all = np.zeros((NCORES, T_pad * P), np.int32)
    dstl_all = np.full((NCORES, T_pad * P), 999.0, np.float32)

    key = (oo * TPT + to) * 2 + ro
    grp_start = np.searchsorted(key, np.arange(NCORES * TPT * 2), side='left')
    grp_end = np.searchsorted(key, np.arange(NCORES * TPT * 2), side='right')
    for c in range(NCORES):
        for t in range(TPT):
            for rem in (0, 1):
                gk = (c * TPT + t) * 2 + rem
                g0, g1 = grp_start[gk], grp_end[gk]
                n_e = g1 - g0
                if n_e == 0:
                    continue
                base = (off[t] + (K_loc[t] if rem else 0)) * P
                s_seg = so[g0:g1]
                if rem == 0:
                    idx1 = idx2 = (s_seg % NPC)
                else:
                    idx1, idx2 = s_table_idx(s_seg), z_table_idx(s_seg)
                srcidx_all[c, base:base + n_e] = idx1.astype(np.int32)
                srcidx2_all[c, base:base + n_e] = idx2.astype(np.int32)
                dstl_all[c, base:base + n_e] = (do_[g0:g1] % NPC - t * P).astype(np.float32)

    # wrap to [128, T_pad]: slot (tile J, lane p) = flat J*128+p -> [p, J]
    srcidx_w = srcidx_all.reshape(NCORES, T_pad, P).transpose(0, 2, 1).copy()
    srcidx2_w = srcidx2_all.reshape(NCORES, T_pad, P).transpose(0, 2, 1).copy()
    dstl_w = dstl_all.reshape(NCORES, T_pad, P).transpose(0, 2, 1).copy()

    # per-core dinv columns [128, TPT] (pad rows -> 1.0)
    dinv_col = np.ones((NCORES, P, TPT), np.float32)
    for c in range(NCORES):
        dv = dinv[c * NPC:(c + 1) * NPC]
        padded = np.concatenate([dv, np.ones(NPCP - NPC, np.float32)])
        dinv_col[c] = padded.reshape(TPT, P).T

    # pooling matrices: B[c][p, t*WG + (g - g_start_c)] = 1/cnt[g]
    gcnt = np.bincount(batch, minlength=int(batch.max()) + 1).astype(np.float32)
    inv_cnt = 1.0 / np.maximum(gcnt, 1.0)
    g_start = np.zeros(NCORES, np.int64)
    Bpool = np.zeros((NCORES, P, TPT * WG), np.float32)
    for c in range(NCORES):
        bb = batch[c * NPC:(c + 1) * NPC]
        g0 = int(bb[0])
        g_start[c] = g0
        rel = bb - g0
        assert rel.max() < WG, f"graph window {WG} exceeded: {rel.max()}"
        node_pos = np.arange(NPC)
        t_idx, p_idx = node_pos // P, node_pos % P
        Bpool[c, p_idx, t_idx * WG + rel] = inv_cnt[bb]

    # x transposed + padded
    xT = np.zeros((NCORES, IN_C, NPCP), np.float32)
    for c in range(NCORES):
        xT[c, :, :NPC] = x[c * NPC:(c + 1) * NPC, :].T

    W1b = np.ascontiguousarray(np.asarray(W1, np.float32))
    W2b = np.ascontiguousarray(np.asarray(W2, np.float32))
    b1f = np.asarray(b1, np.float32).reshape(HID)
    b2f = np.asarray(b2, np.float32).reshape(OUT_C)

    in_maps = []
    for c in range(NCORES):
        in_maps.append({
            "xT": xT[c],
            "W1": W1b,
            "W2": W2b,
            "srcidx": srcidx_w[c],
            "srcidx2": srcidx2_w[c],
            "dstl": dstl_w[c],
            "dinv_col": dinv_col[c],
            "Bpool": Bpool[c],
            "b1": np.tile(b1f, (P, 1)),
            "b2": np.tile(b2f, (P, 1)),
        })
    meta = {
        "K": K.tolist(), "off": off.tolist(), "T_pad": T_pad,
        "K_loc": K_loc.tolist(),
        "g_start": g_start,
        "b1_nz": bool(np.any(b1f != 0)), "b2_nz": bool(np.any(b2f != 0)),
    }
    return in_maps, meta


def _build_program(meta):
    from concourse import bass, mybir
    import concourse.tile as tile

    F32, BF16, I32 = mybir.dt.float32, mybir.dt.bfloat16, mybir.dt.int32
    F32R = mybir.dt.float32r
    AF = mybir.ActivationFunctionType
    K, off, T_pad = meta["K"], meta["off"], meta["T_pad"]
    K_loc = meta["K_loc"]
    b1_nz, b2_nz = meta["b1_nz"], meta["b2_nz"]

    nc = bass.Bass()
    xT = nc.declare_dram_parameter("xT", [IN_C, NPCP], F32, isOutput=False)
    W1 = nc.declare_dram_parameter("W1", [IN_C, HID], F32, isOutput=False)
    W2 = nc.declare_dram_parameter("W2", [HID, OUT_C], F32, isOutput=False)
    srcidx = nc.declare_dram_parameter("srcidx", [P, T_pad], I32, isOutput=False)
    srcidx2 = nc.declare_dram_parameter("srcidx2", [P, T_pad], I32, isOutput=False)
    dstl = nc.declare_dram_parameter("dstl", [P, T_pad], F32, isOutput=False)
    dinv_col = nc.declare_dram_parameter("dinv_col", [P, TPT], F32, isOutput=False)
    Bpool = nc.declare_dram_parameter("Bpool", [P, TPT * WG], F32, isOutput=False)
    b1 = nc.declare_dram_parameter("b1", [P, HID], F32, isOutput=False)
    b2 = nc.declare_dram_parameter("b2", [P, OUT_C], F32, isOutput=False)

    out_nodes = nc.declare_dram_parameter("out_nodes", [NPCP, HID + OUT_C], F32, isOutput=True)
    out_pooled = nc.declare_dram_parameter("out_pooled", [WG, HID + OUT_C], F32, isOutput=True)

    NGRP = -(-NPCP // 512)  # 512-node groups in phase 5
    S_BOUNDS = [0, NPCP]
    Z_BOUNDS = [0, 3584, NPCP]

    with tile.TileContext(nc) as tc:
        with tc.tile_pool(name="const", bufs=1) as cst, \
             tc.tile_pool(name="wpool", bufs=1) as wp, \
             tc.tile_pool(name="xbig", bufs=1) as xbp, \
             tc.tile_pool(name="hstage", bufs=2) as hsp, \
             tc.tile_pool(name="xr", bufs=4) as xrp, \
             tc.tile_pool(name="msg", bufs=8) as msgp, \
             tc.tile_pool(name="s01", bufs=8) as s01p, \
             tc.tile_pool(name="hout", bufs=3) as hp, \
             tc.tile_pool(name="zb", bufs=3) as zbp, \
             tc.tile_pool(name="ps_mm", bufs=2, space="PSUM") as ps_mm, \
             tc.tile_pool(name="ps_tr", bufs=2, space="PSUM") as ps_tr, \
             tc.tile_pool(name="ps_pool", bufs=1, space="PSUM") as ps_pool, \
             tc.tile_pool(name="dram", bufs=1, space="DRAM") as dram:

            # ---- constants in SBUF
            iota_i = cst.tile([P, P], I32)
            nc.gpsimd.iota(iota_i[:], pattern=[[1, P]], base=0, channel_multiplier=0)
            iota_f = cst.tile([P, P], F32)
            nc.vector.tensor_copy(iota_f[:], iota_i[:])
            iota_ci = cst.tile([P, 1], I32)
            nc.gpsimd.iota(iota_ci[:], pattern=[[0, 1]], base=0, channel_multiplier=1)
            iota_cf = cst.tile([P, 1], F32)
            nc.vector.tensor_copy(iota_cf[:], iota_ci[:])
            ident_bf = cst.tile([P, P], F32)
            nc.vector.tensor_scalar(out=ident_bf[:], in0=iota_f[:], scalar1=iota_cf[:, :1],
                                    scalar2=None, op0=mybir.AluOpType.is_equal)

            srcidx_t = cst.tile([P, T_pad], I32)
            nc.sync.dma_start(out=srcidx_t[:], in_=srcidx[:])
            srcidx2_t = cst.tile([P, T_pad], I32)
            nc.sync.dma_start(out=srcidx2_t[:], in_=srcidx2[:])
            dstl_t = cst.tile([P, T_pad], F32)
            nc.sync.dma_start(out=dstl_t[:], in_=dstl[:])
            dinv_t = cst.tile([P, TPT], F32)
            nc.sync.dma_start(out=dinv_t[:], in_=dinv_col[:])
            Bp_t = cst.tile([P, TPT * WG], F32)
            nc.sync.dma_start(out=Bp_t[:], in_=Bpool[:])
            W1_t = wp.tile([P, KCH1, HID], F32)
            W1_r = wp.tile([P, KCH1, HID], F32R)
            for k in range(KCH1):
                nc.sync.dma_start(out=W1_t[:, k, :], in_=W1[k * P:(k + 1) * P, :])
                nc.vector.tensor_copy(W1_r[:, k, :], W1_t[:, k, :])
            W2_t = wp.tile([P, HID // P, OUT_C], F32)
            for k in range(HID // P):
                nc.sync.dma_start(out=W2_t[:, k, :], in_=W2[k * P:(k + 1) * P, :])
            b1_t = cst.tile([P, HID], F32)
            b2_t = cst.tile([P, OUT_C], F32)
            if b1_nz:
                nc.sync.dma_start(out=b1_t[:], in_=b1[:])
            if b2_nz:
                nc.sync.dma_start(out=b2_t[:], in_=b2[:])

            # ---- DRAM scratch
            ag_in_s = dram.tile([NPCP, HID], F32)                        # own s~ (node-major)
            s_table = dram.tile([NCORES * NPCP, HID], F32)
            ag_in_z = dram.tile([NPCP, OUT_C], F32)                      # own z~ (node-major)
            z_table = dram.tile([NCORES * NPCP, OUT_C], F32)

            # ================= phase 1: s~ = dinv * (x @ W1) =================
            halves = [(0, 25), (25, 49)]
            for h0, h1_ in halves:
                ncols = (h1_ - h0) * P
                xb = xbp.tile([P, KCH1, 3200], F32, tag="xb")
                for k in range(KCH1):
                    nc.sync.dma_start(out=xb[:, k, :ncols],
                                      in_=xT[k * P:(k + 1) * P, h0 * P:h1_ * P])
                for t in range(h0, h1_):
                    c0 = (t - h0) * P
                    ps = ps_mm.tile([P, HID], F32, tag="agg", space="PSUM")
                    for k in range(KCH1):
                        xr = xrp.tile([P, P], F32R, tag="xr")
                        nc.vector.tensor_copy(xr[:], xb[:, k, c0:c0 + P])
                        nc.tensor.matmul(out=ps[:], lhsT=xr[:], rhs=W1_r[:, k, :],
                                         start=(k == 0), stop=(k == KCH1 - 1))
                    st = hp.tile([P, HID], F32, tag="st")
                    nc.scalar.activation(st[:], ps[:], AF.Copy, scale=dinv_t[:, t:t + 1])
                    nc.scalar.dma_start(out=ag_in_s[t * P:(t + 1) * P, :], in_=st[:])

            # ================= phase 2: AllGather s~ =================
            nc.gpsimd.collective_compute(
                "AllGather", mybir.AluOpType.bypass,
                replica_groups=[list(range(NCORES))],
                ins=[ag_in_s[:]], outs=[s_table[:]],
            )

            # ================= phase 3: layer-1 aggregation =================
            ps_p1 = ps_pool.tile([WG, HID], F32, space="PSUM")
            for t in range(TPT):
                ps = ps_mm.tile([P, HID], F32, tag="agg", space="PSUM")
                for j in range(off[t], off[t] + K[t]):
                    mt = msgp.tile([P, HID], F32, tag="msg")
                    srcbuf = ag_in_s if (j - off[t]) < K_loc[t] else s_table
                    nc.gpsimd.indirect_dma_start(
                        out=mt[:], out_offset=None,
                        in_=srcbuf[:],
                        in_offset=bass.IndirectOffsetOnAxis(ap=srcidx_t[:, j:j + 1], axis=0),
                    )
                    s01 = s01p.tile([P, P], F32, tag="s01")
                    nc.vector.tensor_scalar(out=s01[:], in0=iota_f[:], scalar1=dstl_t[:, j:j + 1],
                                            scalar2=None, op0=mybir.AluOpType.is_equal)
                    nc.tensor.matmul(out=ps[:], lhsT=s01[:], rhs=mt[:],
                                     start=(j == off[t]), stop=False)
                selfm = msgp.tile([P, HID], F32, tag="msg")
                nc.sync.dma_start(out=selfm[:], in_=ag_in_s[t * P:(t + 1) * P, :])
                nc.tensor.matmul(out=ps[:], lhsT=ident_bf[:], rhs=selfm[:], start=False, stop=True)

                h1 = hp.tile([P, HID], F32, tag="h1")
                if b1_nz:
                    tmp = hp.tile([P, HID], F32, tag="tmp1")
                    nc.scalar.activation(tmp[:], ps[:], AF.Copy, scale=dinv_t[:, t:t + 1])
                    nc.vector.tensor_tensor(out=tmp[:], in0=tmp[:], in1=b1_t[:], op=mybir.AluOpType.add)
                    nc.scalar.activation(h1[:], tmp[:], AF.Relu)
                else:
                    nc.scalar.activation(h1[:], ps[:], AF.Relu, scale=dinv_t[:, t:t + 1])
                nc.scalar.dma_start(out=out_nodes[t * P:(t + 1) * P, 0:HID], in_=h1[:])
                nc.tensor.matmul(out=ps_p1[:], lhsT=Bp_t[:, t * WG:(t + 1) * WG], rhs=h1[:],
                                 start=(t == 0), stop=(t == TPT - 1))
                ht = hp.tile([P, HID], F32, tag="ht")
                nc.scalar.activation(ht[:], h1[:], AF.Copy, scale=dinv_t[:, t:t + 1])
                g, sl = t // 4, t % 4
                if sl == 0:
                    hstage = hsp.tile([P, HID // P, 512], F32, tag="hstage")
                for m in range(HID // P):
                    tp = ps_tr.tile([P, P], F32, tag="tr", space="PSUM")
                    nc.tensor.transpose(out=tp[:], in_=ht[:, m * P:(m + 1) * P], identity=ident_bf[:])
                    nc.vector.tensor_copy(hstage[:, m, sl * P:(sl + 1) * P], tp[:])
                if t == min(4 * g + 3, TPT - 1):
                    w = (sl + 1) * P
                    psz = ps_mm.tile([P, 512], F32, tag="zz", space="PSUM")
                    for chunk in range(HID // P):
                        nc.tensor.matmul(out=psz[:, :w], lhsT=W2_t[:, chunk, :],
                                         rhs=hstage[:, chunk, :w],
                                         start=(chunk == 0), stop=(chunk == HID // P - 1))
                    zb = zbp.tile([P, 512], F32, tag="zb")
                    nc.vector.tensor_copy(zb[:, :w], psz[:, :w])
                    for q in range(w // P):
                        tp2 = ps_tr.tile([P, P], F32, tag="tr", space="PSUM")
                        nc.tensor.transpose(out=tp2[:], in_=zb[:, q * P:(q + 1) * P], identity=ident_bf[:])
                        tb2 = zbp.tile([P, P], F32, tag="zt")
                        nc.vector.tensor_copy(tb2[:], tp2[:])
                        row0 = g * 512 + q * P
                        nc.sync.dma_start(out=ag_in_z[row0:row0 + P, :], in_=tb2[:])
                    if (g + 1) * 512 == Z_BOUNDS[1]:
                        lo, hi = Z_BOUNDS[0], Z_BOUNDS[1]
                        nc.gpsimd.collective_compute(
                            "AllGather", mybir.AluOpType.bypass,
                            replica_groups=[list(range(NCORES))],
                            ins=[ag_in_z[lo:hi, :]],
                            outs=[z_table[NCORES * lo:NCORES * hi, :]],
                        )

            pooled1 = hp.tile([WG, HID], F32, tag="pooled1")
            nc.scalar.activation(pooled1[:], ps_p1[:], AF.Copy)
            nc.sync.dma_start(out=out_pooled[:, 0:HID], in_=pooled1[:])

            # ========= phase 5: AllGather z~ (remaining rows) =========
            lo, hi = Z_BOUNDS[1], Z_BOUNDS[2]
            nc.gpsimd.collective_compute(
                "AllGather", mybir.AluOpType.bypass,
                replica_groups=[list(range(NCORES))],
                ins=[ag_in_z[lo:hi, :]],
                outs=[z_table[NCORES * lo:, :]],
            )

            # ================= phase 6: layer-2 aggregation =================
            ps_p2 = ps_pool.tile([WG, OUT_C], F32, space="PSUM")
            for t in range(TPT):
                ps = ps_mm.tile([P, OUT_C], F32, tag="agg", space="PSUM")
                for j in range(off[t], off[t] + K[t]):
                    mt = msgp.tile([P, OUT_C], F32, tag="msg")
                    srcbuf = ag_in_z if (j - off[t]) < K_loc[t] else z_table
                    nc.gpsimd.indirect_dma_start(
                        out=mt[:], out_offset=None,
                        in_=srcbuf[:],
                        in_offset=bass.IndirectOffsetOnAxis(ap=srcidx2_t[:, j:j + 1], axis=0),
                    )
                    s01 = s01p.tile([P, P], F32, tag="s01")
                    nc.vector.tensor_scalar(out=s01[:], in0=iota_f[:], scalar1=dstl_t[:, j:j + 1],
                                            scalar2=None, op0=mybir.AluOpType.is_equal)
                    nc.tensor.matmul(out=ps[:], lhsT=s01[:], rhs=mt[:],
                                     start=(j == off[t]), stop=False)
                selfm = msgp.tile([P, OUT_C], F32, tag="msg")
                nc.sync.dma_start(out=selfm[:], in_=ag_in_z[t * P:(t + 1) * P, :])
                nc.tensor.matmul(out=ps[:], lhsT=ident_bf[:], rhs=selfm[:], start=False, stop=True)

                h2 = hp.tile([P, OUT_C], F32, tag="h2")
                if b2_nz:
                    tmp = hp.tile([P, OUT_C], F32, tag="tmp2")
                    nc.scalar.activation(tmp[:], ps[:], AF.Copy, scale=dinv_t[:, t:t + 1])
                    nc.vector.tensor_tensor(out=tmp[:], in0=tmp[:], in1=b2_t[:], op=mybir.AluOpType.add)
                    nc.scalar.activation(h2[:], tmp[:], AF.Relu)
                else:
                    nc.scalar.activation(h2[:], ps[:], AF.Relu, scale=dinv_t[:, t:t + 1])
                nc.scalar.dma_start(out=out_nodes[t * P:(t + 1) * P, HID:HID + OUT_C], in_=h2[:])
                nc.tensor.matmul(out=ps_p2[:], lhsT=Bp_t[:, t * WG:(t + 1) * WG], rhs=h2[:],
                                 start=(t == 0), stop=(t == TPT - 1))

            pooled2 = hp.tile([WG, OUT_C], F32, tag="pooled2")
            nc.scalar.activation(pooled2[:], ps_p2[:], AF.Copy)
            nc.sync.dma_start(out=out_pooled[:, HID:HID + OUT_C], in_=pooled2[:])

    _split_multi_waits(nc, mybir)
    return nc


_PROGRAM_CACHE = {}


def kernel(x, edge_index, batch, num_graphs, W1, b1, W2, b2):
    from concourse.bass_utils import run_bass_kernel_spmd

    in_maps, meta = _preprocess(x, edge_index, batch, W1, b1, W2, b2)
    cache_key = (tuple(meta["K"]), tuple(meta["K_loc"]), meta["b1_nz"], meta["b2_nz"])
    nc = _PROGRAM_CACHE.get(cache_key)
    if nc is None:
        nc = _build_program(meta)
        _PROGRAM_CACHE[cache_key] = nc

    res = run_bass_kernel_spmd(nc, in_maps, list(range(NCORES))).results

    G = int(num_graphs)
    node_embed = np.concatenate([res[c]["out_nodes"][:NPC] for c in range(NCORES)], axis=0)
    graph_embed = np.zeros((G + WG, HID + OUT_C), np.float32)
    for c in range(NCORES):
        g0 = int(meta["g_start"][c])
        graph_embed[g0:g0 + WG] += res[c]["out_pooled"]
    graph_embed = graph_embed[:G]
    return graph_embed.astype(np.float32), node_embed.astype(np.float32)


# revision 19
# speedup vs baseline: 1.0016x; 1.0016x over previous
"""GCN encoder (2x GCNConv + ReLU + global mean pool) on 8 Trainium2 cores.

Sharding: nodes (and their incident in-edges) are partitioned across the 8
cores; GCN weights are replicated; per-graph pooled sums are computed as
per-core partials and combined on the host during unsharding.

Self-contained: hardcodes the problem shapes from the task spec.
"""
import sys
sys.path.insert(0, '/opt/trn_rl_repo')

import numpy as np
import ml_dtypes

BF16NP = np.dtype(ml_dtypes.bfloat16)

# Problem shapes (fixed by the task).
N, E, IN_C, HID, OUT_C = 50000, 800000, 768, 256, 128
NCORES = 8
NPC = N // NCORES          # 6250 nodes owned per core
P = 128
TPT = 49                   # dst tiles per core (49*128 = 6272)
NPCP = TPT * P             # padded nodes per core
WG = 64                    # per-core graph window for pooling
KCH1 = IN_C // P           # 6 k-chunks for x @ W1


def _split_multi_waits(nc, mybir):
    """This walrus build rejects >1 sync-wait per instruction; move extra
    waits onto single-wait NoOp carriers inserted just before, same engine."""
    for fn in nc.m.functions:
        for blk in fn.blocks:
            insts = blk.instructions
            pos = 0
            while pos < len(insts):
                inst = insts[pos]
                si = inst.sync_info
                if si is not None and len(si.on_wait) > 1:
                    w = list(si.on_wait)
                    u = list(si.on_update)
                    newds = []
                    for j, wj in enumerate(w[:-1]):
                        d = mybir.InstNoOp(name=f"{inst.name}-sp{j}", ins=[], outs=[])
                        d.engine = inst.engine
                        d.sync_info = mybir.SyncInfo(on_wait=[wj], on_update=[])
                        newds.append(d)
                    inst.sync_info = mybir.SyncInfo(on_wait=[w[-1]], on_update=u)
                    insts[pos:pos] = newds
                    pos += len(newds)
                pos += 1


def _preprocess(x, edge_index, batch, W1, b1, W2, b2):
    """Host-side sharding / index preprocessing. Returns (in_maps, meta)."""
    x = np.asarray(x, np.float32)
    src = np.asarray(edge_index[0], np.int64)
    dst = np.asarray(edge_index[1], np.int64)
    batch = np.asarray(batch, np.int64)

    deg = np.bincount(dst, minlength=N).astype(np.float32) + 1.0
    dinv = 1.0 / np.sqrt(deg)

    owner = dst // NPC
    # Degree-balanced node->slot assignment per core: pack nodes into dst
    # tiles so per-tile edge counts quantize tightly (tiles 0..44 capped at
    # 16*128 edges; tiles 45..48 absorb the overflow). Cuts gather padding.
    ecnt = np.zeros((NCORES, NPC), np.int64)
    np.add.at(ecnt, (owner, dst % NPC), 1)
    slot_of = np.zeros((NCORES, NPC), np.int64)
    node_at = np.full((NCORES, NPCP), -1, np.int64)
    for c in range(NCORES):
        d = ecnt[c]
        order_n = np.argsort(-d, kind='stable')
        bin_deg = np.zeros(TPT, np.int64)
        bin_cnt = np.zeros(TPT, np.int64)
        caps = np.full(TPT, 2046, np.int64)
        caps[45:] = 1 << 40
        for l in order_n:
            ok = (bin_cnt < P) & (bin_deg + d[l] <= caps)
            cand = np.where(ok)[0]
            if len(cand) == 0:
                cand = np.where(bin_cnt < P)[0]
            b = cand[np.argmin(bin_deg[cand])]
            s = b * P + bin_cnt[b]
            slot_of[c, l] = s
            node_at[c, s] = l
            bin_cnt[b] += 1
            bin_deg[b] += d[l]
    dslot = slot_of[owner, dst % NPC]
    tile_of = dslot // P
    is_rem = np.ones(E, bool)   # single padded run per tile (split costs more than it hides)

    # Global node id -> padded table row. The tables are assembled from
    # chunked AllGathers (rank-major within each chunk), so the row layout
    # depends on the chunk boundaries (in local padded rows).
    def make_table_idx(bounds):
        def table_idx(u):
            r = u // NPC
            l = slot_of[r, u % NPC]
            row = np.zeros_like(u)
            base = 0
            for lo, hi in zip(bounds[:-1], bounds[1:]):
                sel = (l >= lo) & (l < hi)
                row[sel] = base + r[sel] * (hi - lo) + (l[sel] - lo)
                base += NCORES * (hi - lo)
            return row
        return table_idx
    S_BOUNDS = [0, NPCP]                # AG1 unsplit (sim: fixed cost per collective dominates)
    Z_BOUNDS = [0, 3584, NPCP]          # AG2 halves (z-groups 0..6 / 7..12)
    s_table_idx = make_table_idx(S_BOUNDS)
    z_table_idx = make_table_idx(Z_BOUNDS)


    # per-(core, tile) local/remote counts -> shared padded schedule
    cnt_loc = np.zeros((NCORES, TPT), np.int64)
    cnt_rem = np.zeros((NCORES, TPT), np.int64)
    np.add.at(cnt_loc, (owner[~is_rem], tile_of[~is_rem]), 1)
    np.add.at(cnt_rem, (owner[is_rem], tile_of[is_rem]), 1)
    K_loc = -(-cnt_loc.max(axis=0) // P)
    K_rem = np.maximum(1, -(-cnt_rem.max(axis=0) // P))
    K = K_loc + K_rem
    off = np.concatenate([[0], np.cumsum(K)])[:-1]
    T_pad = int(K.sum())

    # order edges by (owner, tile, remote?, dst_local) and fill padded slots
    order = np.lexsort((dst, is_rem, tile_of, owner))
    so, do_, oo, to, ro = src[order], dst[order], owner[order], tile_of[order], is_rem[order]

    srcidx_all = np.zeros((NCORES, T_pad * P), np.int32)
    srcidx2_all = np.zeros((NCORES, T_pad * P), np.int32)
    dstl_all = np.full((NCORES, T_pad * P), 999.0, np.float32)

    key = (oo * TPT + to) * 2 + ro
    grp_start = np.searchsorted(key, np.arange(NCORES * TPT * 2), side='left')
    grp_end = np.searchsorted(key, np.arange(NCORES * TPT * 2), side='right')
    for c in range(NCORES):
        for t in range(TPT):
            for rem in (0, 1):
                gk = (c * TPT + t) * 2 + rem
                g0, g1 = grp_start[gk], grp_end[gk]
                n_e = g1 - g0
                if n_e == 0:
                    continue
                base = (off[t] + (K_loc[t] if rem else 0)) * P
                s_seg = so[g0:g1]
                if rem == 0:
                    idx1 = idx2 = slot_of[c, s_seg % NPC]
                else:
                    idx1, idx2 = s_table_idx(s_seg), z_table_idx(s_seg)
                srcidx_all[c, base:base + n_e] = idx1.astype(np.int32)
                srcidx2_all[c, base:base + n_e] = idx2.astype(np.int32)
                dstl_all[c, base:base + n_e] = (slot_of[c, do_[g0:g1] % NPC] - t * P).astype(np.float32)

    # wrap to [128, T_pad]: slot (tile J, lane p) = flat J*128+p -> [p, J]
    srcidx_w = srcidx_all.reshape(NCORES, T_pad, P).transpose(0, 2, 1).copy()
    srcidx2_w = srcidx2_all.reshape(NCORES, T_pad, P).transpose(0, 2, 1).copy()
    dstl_w = dstl_all.reshape(NCORES, T_pad, P).transpose(0, 2, 1).copy()

    # per-core dinv columns [128, TPT] in slot order (pad slots -> 1.0)
    dinv_col = np.ones((NCORES, P, TPT), np.float32)
    for c in range(NCORES):
        padded = np.ones(NPCP, np.float32)
        real = node_at[c] >= 0
        padded[real] = dinv[c * NPC + node_at[c][real]]
        dinv_col[c] = padded.reshape(TPT, P).T

    # pooling matrices: B[c][p, t*WG + (g - g_start_c)] = 1/cnt[g]
    gcnt = np.bincount(batch, minlength=int(batch.max()) + 1).astype(np.float32)
    inv_cnt = 1.0 / np.maximum(gcnt, 1.0)
    g_start = np.zeros(NCORES, np.int64)
    Bpool = np.zeros((NCORES, P, TPT * WG), np.float32)
    for c in range(NCORES):
        bb = batch[c * NPC:(c + 1) * NPC]
        g0 = int(bb[0])
        g_start[c] = g0
        rel = bb - g0
        assert rel.max() < WG, f"graph window {WG} exceeded: {rel.max()}"
        spos = slot_of[c]
        t_idx, p_idx = spos // P, spos % P
        Bpool[c, p_idx, t_idx * WG + rel] = inv_cnt[bb]

    # x transposed + padded, columns in slot order
    xT = np.zeros((NCORES, IN_C, NPCP), np.float32)
    for c in range(NCORES):
        xT[c, :, slot_of[c]] = x[c * NPC:(c + 1) * NPC, :].T.transpose(1, 0)

    W1b = np.ascontiguousarray(np.asarray(W1, np.float32))
    W2b = np.ascontiguousarray(np.asarray(W2, np.float32))
    b1f = np.asarray(b1, np.float32).reshape(HID)
    b2f = np.asarray(b2, np.float32).reshape(OUT_C)

    in_maps = []
    for c in range(NCORES):
        in_maps.append({
            "xT": xT[c],
            "W1": W1b,
            "W2": W2b,
            "srcidx": srcidx_w[c],
            "srcidx2": srcidx2_w[c],
            "dstl": dstl_w[c],
            "dinv_col": dinv_col[c],
            "Bpool": Bpool[c],
            "b1": np.tile(b1f, (P, 1)),
            "b2": np.tile(b2f, (P, 1)),
        })
    meta = {
        "K": K.tolist(), "off": off.tolist(), "T_pad": T_pad,
        "K_loc": K_loc.tolist(),
        "g_start": g_start, "slot_of": slot_of,
        "b1_nz": bool(np.any(b1f != 0)), "b2_nz": bool(np.any(b2f != 0)),
    }
    return in_maps, meta


def _build_program(meta):
    from concourse import bass, mybir
    import concourse.tile as tile

    F32, BF16, I32 = mybir.dt.float32, mybir.dt.bfloat16, mybir.dt.int32
    F32R = mybir.dt.float32r
    AF = mybir.ActivationFunctionType
    K, off, T_pad = meta["K"], meta["off"], meta["T_pad"]
    K_loc = meta["K_loc"]
    b1_nz, b2_nz = meta["b1_nz"], meta["b2_nz"]

    nc = bass.Bass()
    xT = nc.declare_dram_parameter("xT", [IN_C, NPCP], F32, isOutput=False)
    W1 = nc.declare_dram_parameter("W1", [IN_C, HID], F32, isOutput=False)
    W2 = nc.declare_dram_parameter("W2", [HID, OUT_C], F32, isOutput=False)
    srcidx = nc.declare_dram_parameter("srcidx", [P, T_pad], I32, isOutput=False)
    srcidx2 = nc.declare_dram_parameter("srcidx2", [P, T_pad], I32, isOutput=False)
    dstl = nc.declare_dram_parameter("dstl", [P, T_pad], F32, isOutput=False)
    dinv_col = nc.declare_dram_parameter("dinv_col", [P, TPT], F32, isOutput=False)
    Bpool = nc.declare_dram_parameter("Bpool", [P, TPT * WG], F32, isOutput=False)
    b1 = nc.declare_dram_parameter("b1", [P, HID], F32, isOutput=False)
    b2 = nc.declare_dram_parameter("b2", [P, OUT_C], F32, isOutput=False)

    out_nodes = nc.declare_dram_parameter("out_nodes", [NPCP, HID + OUT_C], F32, isOutput=True)
    out_pooled = nc.declare_dram_parameter("out_pooled", [WG, HID + OUT_C], F32, isOutput=True)

    NGRP = -(-NPCP // 512)  # 512-node groups in phase 5
    S_BOUNDS = [0, NPCP]
    Z_BOUNDS = [0, 3584, NPCP]

    with tile.TileContext(nc) as tc:
        with tc.tile_pool(name="const", bufs=1) as cst, \
             tc.tile_pool(name="wpool", bufs=1) as wp, \
             tc.tile_pool(name="xbig", bufs=1) as xbp, \
             tc.tile_pool(name="hstage", bufs=2) as hsp, \
             tc.tile_pool(name="xr", bufs=4) as xrp, \
             tc.tile_pool(name="msg", bufs=8) as msgp, \
             tc.tile_pool(name="s01", bufs=8) as s01p, \
             tc.tile_pool(name="hout", bufs=3) as hp, \
             tc.tile_pool(name="zb", bufs=3) as zbp, \
             tc.tile_pool(name="ps_mm", bufs=2, space="PSUM") as ps_mm, \
             tc.tile_pool(name="ps_tr", bufs=2, space="PSUM") as ps_tr, \
             tc.tile_pool(name="ps_pool", bufs=1, space="PSUM") as ps_pool, \
             tc.tile_pool(name="dram", bufs=1, space="DRAM") as dram:

            # ---- constants in SBUF
            iota_i = cst.tile([P, P], I32)
            nc.gpsimd.iota(iota_i[:], pattern=[[1, P]], base=0, channel_multiplier=0)
            iota_f = cst.tile([P, P], F32)
            nc.vector.tensor_copy(iota_f[:], iota_i[:])
            iota_ci = cst.tile([P, 1], I32)
            nc.gpsimd.iota(iota_ci[:], pattern=[[0, 1]], base=0, channel_multiplier=1)
            iota_cf = cst.tile([P, 1], F32)
            nc.vector.tensor_copy(iota_cf[:], iota_ci[:])
            ident_bf = cst.tile([P, P], F32)
            nc.vector.tensor_scalar(out=ident_bf[:], in0=iota_f[:], scalar1=iota_cf[:, :1],
                                    scalar2=None, op0=mybir.AluOpType.is_equal)

            srcidx_t = cst.tile([P, T_pad], I32)
            nc.sync.dma_start(out=srcidx_t[:], in_=srcidx[:])
            srcidx2_t = cst.tile([P, T_pad], I32)
            nc.sync.dma_start(out=srcidx2_t[:], in_=srcidx2[:])
            dstl_t = cst.tile([P, T_pad], F32)
            nc.sync.dma_start(out=dstl_t[:], in_=dstl[:])
            dinv_t = cst.tile([P, TPT], F32)
            nc.sync.dma_start(out=dinv_t[:], in_=dinv_col[:])
            Bp_t = cst.tile([P, TPT * WG], F32)
            nc.sync.dma_start(out=Bp_t[:], in_=Bpool[:])
            W1_t = wp.tile([P, KCH1, HID], F32)
            W1_r = wp.tile([P, KCH1, HID], F32R)
            for k in range(KCH1):
                nc.sync.dma_start(out=W1_t[:, k, :], in_=W1[k * P:(k + 1) * P, :])
                nc.vector.tensor_copy(W1_r[:, k, :], W1_t[:, k, :])
            W2_t = wp.tile([P, HID // P, OUT_C], F32)
            for k in range(HID // P):
                nc.sync.dma_start(out=W2_t[:, k, :], in_=W2[k * P:(k + 1) * P, :])
            b1_t = cst.tile([P, HID], F32)
            b2_t = cst.tile([P, OUT_C], F32)
            if b1_nz:
                nc.sync.dma_start(out=b1_t[:], in_=b1[:])
            if b2_nz:
                nc.sync.dma_start(out=b2_t[:], in_=b2[:])

            # ---- DRAM scratch
            ag_in_s = dram.tile([NPCP, HID], F32)                        # own s~ (node-major)
            s_table = dram.tile([NCORES * NPCP, HID], F32)
            ag_in_z = dram.tile([NPCP, OUT_C], F32)                      # own z~ (node-major)
            z_table = dram.tile([NCORES * NPCP, OUT_C], F32)

            # ================= phase 1: s~ = dinv * (x @ W1) =================
            halves = [(0, 25), (25, 49)]
            for h0, h1_ in halves:
                ncols = (h1_ - h0) * P
                xb = xbp.tile([P, KCH1, 3200], F32, tag="xb")
                for k in range(KCH1):
                    nc.sync.dma_start(out=xb[:, k, :ncols],
                                      in_=xT[k * P:(k + 1) * P, h0 * P:h1_ * P])
                for t in range(h0, h1_):
                    c0 = (t - h0) * P
                    ps = ps_mm.tile([P, HID], F32, tag="agg", space="PSUM")
                    for k in range(KCH1):
                        xr = xrp.tile([P, P], F32R, tag="xr")
                        nc.vector.tensor_copy(xr[:], xb[:, k, c0:c0 + P])
                        nc.tensor.matmul(out=ps[:], lhsT=xr[:], rhs=W1_r[:, k, :],
                                         start=(k == 0), stop=(k == KCH1 - 1))
                    st = hp.tile([P, HID], F32, tag="st")
                    nc.scalar.activation(st[:], ps[:], AF.Copy, scale=dinv_t[:, t:t + 1])
                    nc.scalar.dma_start(out=ag_in_s[t * P:(t + 1) * P, :], in_=st[:])

            # ================= phase 2: AllGather s~ =================
            nc.gpsimd.collective_compute(
                "AllGather", mybir.AluOpType.bypass,
                replica_groups=[list(range(NCORES))],
                ins=[ag_in_s[:]], outs=[s_table[:]],
            )

            # ================= phase 3: layer-1 aggregation =================
            ps_p1 = ps_pool.tile([WG, HID], F32, space="PSUM")
            for t in range(TPT):
                ps = ps_mm.tile([P, HID], F32, tag="agg", space="PSUM")
                for j in range(off[t], off[t] + K[t]):
                    mt = msgp.tile([P, HID], F32, tag="msg")
                    srcbuf = ag_in_s if (j - off[t]) < K_loc[t] else s_table
                    nc.gpsimd.indirect_dma_start(
                        out=mt[:], out_offset=None,
                        in_=srcbuf[:],
                        in_offset=bass.IndirectOffsetOnAxis(ap=srcidx_t[:, j:j + 1], axis=0),
                    )
                    s01 = s01p.tile([P, P], F32, tag="s01")
                    nc.vector.tensor_scalar(out=s01[:], in0=iota_f[:], scalar1=dstl_t[:, j:j + 1],
                                            scalar2=None, op0=mybir.AluOpType.is_equal)
                    nc.tensor.matmul(out=ps[:], lhsT=s01[:], rhs=mt[:],
                                     start=(j == off[t]), stop=False)
                selfm = msgp.tile([P, HID], F32, tag="msg")
                nc.sync.dma_start(out=selfm[:], in_=ag_in_s[t * P:(t + 1) * P, :])
                nc.tensor.matmul(out=ps[:], lhsT=ident_bf[:], rhs=selfm[:], start=False, stop=True)

                h1 = hp.tile([P, HID], F32, tag="h1")
                if b1_nz:
                    tmp = hp.tile([P, HID], F32, tag="tmp1")
                    nc.scalar.activation(tmp[:], ps[:], AF.Copy, scale=dinv_t[:, t:t + 1])
                    nc.vector.tensor_tensor(out=tmp[:], in0=tmp[:], in1=b1_t[:], op=mybir.AluOpType.add)
                    nc.scalar.activation(h1[:], tmp[:], AF.Relu)
                else:
                    nc.scalar.activation(h1[:], ps[:], AF.Relu, scale=dinv_t[:, t:t + 1])
                nc.scalar.dma_start(out=out_nodes[t * P:(t + 1) * P, 0:HID], in_=h1[:])
                nc.tensor.matmul(out=ps_p1[:], lhsT=Bp_t[:, t * WG:(t + 1) * WG], rhs=h1[:],
                                 start=(t == 0), stop=(t == TPT - 1))
                ht = hp.tile([P, HID], F32, tag="ht")
                nc.scalar.activation(ht[:], h1[:], AF.Copy, scale=dinv_t[:, t:t + 1])
                g, sl = t // 4, t % 4
                if sl == 0:
                    hstage = hsp.tile([P, HID // P, 512], F32, tag="hstage")
                for m in range(HID // P):
                    tp = ps_tr.tile([P, P], F32, tag="tr", space="PSUM")
                    nc.tensor.transpose(out=tp[:], in_=ht[:, m * P:(m + 1) * P], identity=ident_bf[:])
                    nc.vector.tensor_copy(hstage[:, m, sl * P:(sl + 1) * P], tp[:])
                if t == min(4 * g + 3, TPT - 1):
                    w = (sl + 1) * P
                    psz = ps_mm.tile([P, 512], F32, tag="zz", space="PSUM")
                    for chunk in range(HID // P):
                        nc.tensor.matmul(out=psz[:, :w], lhsT=W2_t[:, chunk, :],
                                         rhs=hstage[:, chunk, :w],
                                         start=(chunk == 0), stop=(chunk == HID // P - 1))
                    zb = zbp.tile([P, 512], F32, tag="zb")
                    nc.vector.tensor_copy(zb[:, :w], psz[:, :w])
                    for q in range(w // P):
                        tp2 = ps_tr.tile([P, P], F32, tag="tr", space="PSUM")
                        nc.tensor.transpose(out=tp2[:], in_=zb[:, q * P:(q + 1) * P], identity=ident_bf[:])
                        tb2 = zbp.tile([P, P], F32, tag="zt")
                        nc.vector.tensor_copy(tb2[:], tp2[:])
                        row0 = g * 512 + q * P
                        nc.sync.dma_start(out=ag_in_z[row0:row0 + P, :], in_=tb2[:])
                    if (g + 1) * 512 == Z_BOUNDS[1]:
                        lo, hi = Z_BOUNDS[0], Z_BOUNDS[1]
                        nc.gpsimd.collective_compute(
                            "AllGather", mybir.AluOpType.bypass,
                            replica_groups=[list(range(NCORES))],
                            ins=[ag_in_z[lo:hi, :]],
                            outs=[z_table[NCORES * lo:NCORES * hi, :]],
                        )

            pooled1 = hp.tile([WG, HID], F32, tag="pooled1")
            nc.scalar.activation(pooled1[:], ps_p1[:], AF.Copy)
            nc.sync.dma_start(out=out_pooled[:, 0:HID], in_=pooled1[:])

            # ========= phase 5: AllGather z~ (remaining rows) =========
            lo, hi = Z_BOUNDS[1], Z_BOUNDS[2]
            nc.gpsimd.collective_compute(
                "AllGather", mybir.AluOpType.bypass,
                replica_groups=[list(range(NCORES))],
                ins=[ag_in_z[lo:hi, :]],
                outs=[z_table[NCORES * lo:, :]],
            )

            # ================= phase 6: layer-2 aggregation =================
            ps_p2 = ps_pool.tile([WG, OUT_C], F32, space="PSUM")
            for t in range(TPT):
                ps = ps_mm.tile([P, OUT_C], F32, tag="agg", space="PSUM")
                for j in range(off[t], off[t] + K[t]):
                    mt = msgp.tile([P, OUT_C], F32, tag="msg")
                    srcbuf = ag_in_z if (j - off[t]) < K_loc[t] else z_table
                    nc.gpsimd.indirect_dma_start(
                        out=mt[:], out_offset=None,
                        in_=srcbuf[:],
                        in_offset=bass.IndirectOffsetOnAxis(ap=srcidx2_t[:, j:j + 1], axis=0),
                    )
                    s01 = s01p.tile([P, P], F32, tag="s01")
                    nc.vector.tensor_scalar(out=s01[:], in0=iota_f[:], scalar1=dstl_t[:, j:j + 1],
                                            scalar2=None, op0=mybir.AluOpType.is_equal)
                    nc.tensor.matmul(out=ps[:], lhsT=s01[:], rhs=mt[:],
                                     start=(j == off[t]), stop=False)
                selfm = msgp.tile([P, OUT_C], F32, tag="msg")
                nc.sync.dma_start(out=selfm[:], in_=ag_in_z[t * P:(t + 1) * P, :])
                nc.tensor.matmul(out=ps[:], lhsT=ident_bf[:], rhs=selfm[:], start=False, stop=True)

                h2 = hp.tile([P, OUT_C], F32, tag="h2")
                if b2_nz:
                    tmp = hp.tile([P, OUT_C], F32, tag="tmp2")
                    nc.scalar.activation(tmp[:], ps[:], AF.Copy, scale=dinv_t[:, t:t + 1])
                    nc.vector.tensor_tensor(out=tmp[:], in0=tmp[:], in1=b2_t[:], op=mybir.AluOpType.add)
                    nc.scalar.activation(h2[:], tmp[:], AF.Relu)
                else:
                    nc.scalar.activation(h2[:], ps[:], AF.Relu, scale=dinv_t[:, t:t + 1])
                nc.scalar.dma_start(out=out_nodes[t * P:(t + 1) * P, HID:HID + OUT_C], in_=h2[:])
                nc.tensor.matmul(out=ps_p2[:], lhsT=Bp_t[:, t * WG:(t + 1) * WG], rhs=h2[:],
                                 start=(t == 0), stop=(t == TPT - 1))

            pooled2 = hp.tile([WG, OUT_C], F32, tag="pooled2")
            nc.scalar.activation(pooled2[:], ps_p2[:], AF.Copy)
            nc.sync.dma_start(out=out_pooled[:, HID:HID + OUT_C], in_=pooled2[:])

    _split_multi_waits(nc, mybir)
    return nc


_PROGRAM_CACHE = {}


def kernel(x, edge_index, batch, num_graphs, W1, b1, W2, b2):
    from concourse.bass_utils import run_bass_kernel_spmd

    in_maps, meta = _preprocess(x, edge_index, batch, W1, b1, W2, b2)
    cache_key = (tuple(meta["K"]), tuple(meta["K_loc"]), meta["b1_nz"], meta["b2_nz"])
    nc = _PROGRAM_CACHE.get(cache_key)
    if nc is None:
        nc = _build_program(meta)
        _PROGRAM_CACHE[cache_key] = nc

    res = run_bass_kernel_spmd(nc, in_maps, list(range(NCORES))).results

    G = int(num_graphs)
    node_embed = np.concatenate(
        [res[c]["out_nodes"][meta["slot_of"][c]] for c in range(NCORES)], axis=0)
    graph_embed = np.zeros((G + WG, HID + OUT_C), np.float32)
    for c in range(NCORES):
        g0 = int(meta["g_start"][c])
        graph_embed[g0:g0 + WG] += res[c]["out_pooled"]
    graph_embed = graph_embed[:G]
    return graph_embed.astype(np.float32), node_embed.astype(np.float32)


# revision 20
# speedup vs baseline: 1.0178x; 1.0162x over previous
"""GCN encoder (2x GCNConv + ReLU + global mean pool) on 8 Trainium2 cores.

Sharding: nodes (and their incident in-edges) are partitioned across the 8
cores; GCN weights are replicated; per-graph pooled sums are computed as
per-core partials and combined on the host during unsharding.

Self-contained: hardcodes the problem shapes from the task spec.
"""
import sys
sys.path.insert(0, '/opt/trn_rl_repo')

import numpy as np
import ml_dtypes

BF16NP = np.dtype(ml_dtypes.bfloat16)

# Problem shapes (fixed by the task).
N, E, IN_C, HID, OUT_C = 50000, 800000, 768, 256, 128
NCORES = 8
NPC = N // NCORES          # 6250 nodes owned per core
P = 128
TPT = 49                   # dst tiles per core (49*128 = 6272)
NPCP = TPT * P             # padded nodes per core
WG = 64                    # per-core graph window for pooling
KCH1 = IN_C // P           # 6 k-chunks for x @ W1


def _split_multi_waits(nc, mybir):
    """This walrus build rejects >1 sync-wait per instruction; move extra
    waits onto single-wait NoOp carriers inserted just before, same engine."""
    for fn in nc.m.functions:
        for blk in fn.blocks:
            insts = blk.instructions
            pos = 0
            while pos < len(insts):
                inst = insts[pos]
                si = inst.sync_info
                if si is not None and len(si.on_wait) > 1:
                    w = list(si.on_wait)
                    u = list(si.on_update)
                    newds = []
                    for j, wj in enumerate(w[:-1]):
                        d = mybir.InstNoOp(name=f"{inst.name}-sp{j}", ins=[], outs=[])
                        d.engine = inst.engine
                        d.sync_info = mybir.SyncInfo(on_wait=[wj], on_update=[])
                        newds.append(d)
                    inst.sync_info = mybir.SyncInfo(on_wait=[w[-1]], on_update=u)
                    insts[pos:pos] = newds
                    pos += len(newds)
                pos += 1


def _preprocess(x, edge_index, batch, W1, b1, W2, b2):
    """Host-side sharding / index preprocessing. Returns (in_maps, meta)."""
    x = np.asarray(x, np.float32)
    src = np.asarray(edge_index[0], np.int64)
    dst = np.asarray(edge_index[1], np.int64)
    batch = np.asarray(batch, np.int64)

    deg = np.bincount(dst, minlength=N).astype(np.float32) + 1.0
    dinv = 1.0 / np.sqrt(deg)

    owner = dst // NPC
    # Degree-balanced node->slot assignment per core: pack nodes into dst
    # tiles so per-tile edge counts quantize tightly (tiles 0..44 capped at
    # 16*128 edges; tiles 45..48 absorb the overflow). Cuts gather padding.
    ecnt = np.zeros((NCORES, NPC), np.int64)
    np.add.at(ecnt, (owner, dst % NPC), 1)
    slot_of = np.zeros((NCORES, NPC), np.int64)
    node_at = np.full((NCORES, NPCP), -1, np.int64)
    for c in range(NCORES):
        d = ecnt[c]
        order_n = np.argsort(-d, kind='stable')
        bin_deg = np.zeros(TPT, np.int64)
        bin_cnt = np.zeros(TPT, np.int64)
        caps = np.full(TPT, 16 * P, np.int64)
        caps[45:] = 1 << 40
        for l in order_n:
            ok = (bin_cnt < P) & (bin_deg + d[l] <= caps)
            cand = np.where(ok[:45])[0]          # prefer capped bins
            if len(cand) == 0:
                cand = np.where(ok)[0]           # then overflow bins
            if len(cand) == 0:
                cand = np.where(bin_cnt < P)[0]  # last resort: any slot
            b = cand[np.argmin(bin_deg[cand])]
            s = b * P + bin_cnt[b]
            slot_of[c, l] = s
            node_at[c, s] = l
            bin_cnt[b] += 1
            bin_deg[b] += d[l]
    dslot = slot_of[owner, dst % NPC]
    tile_of = dslot // P
    is_rem = np.ones(E, bool)   # single padded run per tile (split costs more than it hides)

    # Global node id -> padded table row. The tables are assembled from
    # chunked AllGathers (rank-major within each chunk), so the row layout
    # depends on the chunk boundaries (in local padded rows).
    def make_table_idx(bounds):
        def table_idx(u):
            r = u // NPC
            l = slot_of[r, u % NPC]
            row = np.zeros_like(u)
            base = 0
            for lo, hi in zip(bounds[:-1], bounds[1:]):
                sel = (l >= lo) & (l < hi)
                row[sel] = base + r[sel] * (hi - lo) + (l[sel] - lo)
                base += NCORES * (hi - lo)
            return row
        return table_idx
    S_BOUNDS = [0, NPCP]                # AG1 unsplit (sim: fixed cost per collective dominates)
    Z_BOUNDS = [0, 3584, NPCP]          # AG2 halves (z-groups 0..6 / 7..12)
    s_table_idx = make_table_idx(S_BOUNDS)
    z_table_idx = make_table_idx(Z_BOUNDS)


    # per-(core, tile) local/remote counts -> shared padded schedule
    cnt_loc = np.zeros((NCORES, TPT), np.int64)
    cnt_rem = np.zeros((NCORES, TPT), np.int64)
    np.add.at(cnt_loc, (owner[~is_rem], tile_of[~is_rem]), 1)
    np.add.at(cnt_rem, (owner[is_rem], tile_of[is_rem]), 1)
    K_loc = -(-cnt_loc.max(axis=0) // P)
    K_rem = np.maximum(1, -(-cnt_rem.max(axis=0) // P))
    K = K_loc + K_rem
    off = np.concatenate([[0], np.cumsum(K)])[:-1]
    T_pad = int(K.sum())

    # order edges by (owner, tile, remote?, dst_local) and fill padded slots
    order = np.lexsort((dst, is_rem, tile_of, owner))
    so, do_, oo, to, ro = src[order], dst[order], owner[order], tile_of[order], is_rem[order]

    srcidx_all = np.zeros((NCORES, T_pad * P), np.int32)
    srcidx2_all = np.zeros((NCORES, T_pad * P), np.int32)
    dstl_all = np.full((NCORES, T_pad * P), 999.0, np.float32)

    key = (oo * TPT + to) * 2 + ro
    grp_start = np.searchsorted(key, np.arange(NCORES * TPT * 2), side='left')
    grp_end = np.searchsorted(key, np.arange(NCORES * TPT * 2), side='right')
    for c in range(NCORES):
        for t in range(TPT):
            for rem in (0, 1):
                gk = (c * TPT + t) * 2 + rem
                g0, g1 = grp_start[gk], grp_end[gk]
                n_e = g1 - g0
                if n_e == 0:
                    continue
                base = (off[t] + (K_loc[t] if rem else 0)) * P
                s_seg = so[g0:g1]
                if rem == 0:
                    idx1 = idx2 = slot_of[c, s_seg % NPC]
                else:
                    idx1, idx2 = s_table_idx(s_seg), z_table_idx(s_seg)
                srcidx_all[c, base:base + n_e] = idx1.astype(np.int32)
                srcidx2_all[c, base:base + n_e] = idx2.astype(np.int32)
                dstl_all[c, base:base + n_e] = (slot_of[c, do_[g0:g1] % NPC] - t * P).astype(np.float32)

    # wrap to [128, T_pad]: slot (tile J, lane p) = flat J*128+p -> [p, J]
    srcidx_w = srcidx_all.reshape(NCORES, T_pad, P).transpose(0, 2, 1).copy()
    srcidx2_w = srcidx2_all.reshape(NCORES, T_pad, P).transpose(0, 2, 1).copy()
    dstl_w = dstl_all.reshape(NCORES, T_pad, P).transpose(0, 2, 1).copy()

    # per-core dinv columns [128, TPT] in slot order (pad slots -> 1.0)
    dinv_col = np.ones((NCORES, P, TPT), np.float32)
    for c in range(NCORES):
        padded = np.ones(NPCP, np.float32)
        real = node_at[c] >= 0
        padded[real] = dinv[c * NPC + node_at[c][real]]
        dinv_col[c] = padded.reshape(TPT, P).T

    # pooling matrices: B[c][p, t*WG + (g - g_start_c)] = 1/cnt[g]
    gcnt = np.bincount(batch, minlength=int(batch.max()) + 1).astype(np.float32)
    inv_cnt = 1.0 / np.maximum(gcnt, 1.0)
    g_start = np.zeros(NCORES, np.int64)
    Bpool = np.zeros((NCORES, P, TPT * WG), np.float32)
    for c in range(NCORES):
        bb = batch[c * NPC:(c + 1) * NPC]
        g0 = int(bb[0])
        g_start[c] = g0
        rel = bb - g0
        assert rel.max() < WG, f"graph window {WG} exceeded: {rel.max()}"
        spos = slot_of[c]
        t_idx, p_idx = spos // P, spos % P
        Bpool[c, p_idx, t_idx * WG + rel] = inv_cnt[bb]

    # x transposed + padded, columns in slot order
    xT = np.zeros((NCORES, IN_C, NPCP), np.float32)
    for c in range(NCORES):
        xT[c, :, slot_of[c]] = x[c * NPC:(c + 1) * NPC, :].T.transpose(1, 0)

    W1b = np.ascontiguousarray(np.asarray(W1, np.float32))
    W2b = np.ascontiguousarray(np.asarray(W2, np.float32))
    b1f = np.asarray(b1, np.float32).reshape(HID)
    b2f = np.asarray(b2, np.float32).reshape(OUT_C)

    in_maps = []
    for c in range(NCORES):
        in_maps.append({
            "xT": xT[c],
            "W1": W1b,
            "W2": W2b,
            "srcidx": srcidx_w[c],
            "srcidx2": srcidx2_w[c],
            "dstl": dstl_w[c],
            "dinv_col": dinv_col[c],
            "Bpool": Bpool[c],
            "b1": np.tile(b1f, (P, 1)),
            "b2": np.tile(b2f, (P, 1)),
        })
    meta = {
        "K": K.tolist(), "off": off.tolist(), "T_pad": T_pad,
        "K_loc": K_loc.tolist(),
        "g_start": g_start, "slot_of": slot_of,
        "b1_nz": bool(np.any(b1f != 0)), "b2_nz": bool(np.any(b2f != 0)),
    }
    return in_maps, meta


def _build_program(meta):
    from concourse import bass, mybir
    import concourse.tile as tile

    F32, BF16, I32 = mybir.dt.float32, mybir.dt.bfloat16, mybir.dt.int32
    F32R = mybir.dt.float32r
    AF = mybir.ActivationFunctionType
    K, off, T_pad = meta["K"], meta["off"], meta["T_pad"]
    K_loc = meta["K_loc"]
    b1_nz, b2_nz = meta["b1_nz"], meta["b2_nz"]

    nc = bass.Bass()
    xT = nc.declare_dram_parameter("xT", [IN_C, NPCP], F32, isOutput=False)
    W1 = nc.declare_dram_parameter("W1", [IN_C, HID], F32, isOutput=False)
    W2 = nc.declare_dram_parameter("W2", [HID, OUT_C], F32, isOutput=False)
    srcidx = nc.declare_dram_parameter("srcidx", [P, T_pad], I32, isOutput=False)
    srcidx2 = nc.declare_dram_parameter("srcidx2", [P, T_pad], I32, isOutput=False)
    dstl = nc.declare_dram_parameter("dstl", [P, T_pad], F32, isOutput=False)
    dinv_col = nc.declare_dram_parameter("dinv_col", [P, TPT], F32, isOutput=False)
    Bpool = nc.declare_dram_parameter("Bpool", [P, TPT * WG], F32, isOutput=False)
    b1 = nc.declare_dram_parameter("b1", [P, HID], F32, isOutput=False)
    b2 = nc.declare_dram_parameter("b2", [P, OUT_C], F32, isOutput=False)

    out_nodes = nc.declare_dram_parameter("out_nodes", [NPCP, HID + OUT_C], F32, isOutput=True)
    out_pooled = nc.declare_dram_parameter("out_pooled", [WG, HID + OUT_C], F32, isOutput=True)

    NGRP = -(-NPCP // 512)  # 512-node groups in phase 5
    S_BOUNDS = [0, NPCP]
    Z_BOUNDS = [0, 3584, NPCP]

    with tile.TileContext(nc) as tc:
        with tc.tile_pool(name="const", bufs=1) as cst, \
             tc.tile_pool(name="wpool", bufs=1) as wp, \
             tc.tile_pool(name="xbig", bufs=1) as xbp, \
             tc.tile_pool(name="hstage", bufs=2) as hsp, \
             tc.tile_pool(name="xr", bufs=4) as xrp, \
             tc.tile_pool(name="msg", bufs=8) as msgp, \
             tc.tile_pool(name="s01", bufs=8) as s01p, \
             tc.tile_pool(name="hout", bufs=3) as hp, \
             tc.tile_pool(name="zb", bufs=3) as zbp, \
             tc.tile_pool(name="ps_mm", bufs=2, space="PSUM") as ps_mm, \
             tc.tile_pool(name="ps_tr", bufs=2, space="PSUM") as ps_tr, \
             tc.tile_pool(name="ps_pool", bufs=1, space="PSUM") as ps_pool, \
             tc.tile_pool(name="dram", bufs=1, space="DRAM") as dram:

            # ---- constants in SBUF
            iota_i = cst.tile([P, P], I32)
            nc.gpsimd.iota(iota_i[:], pattern=[[1, P]], base=0, channel_multiplier=0)
            iota_f = cst.tile([P, P], F32)
            nc.vector.tensor_copy(iota_f[:], iota_i[:])
            iota_ci = cst.tile([P, 1], I32)
            nc.gpsimd.iota(iota_ci[:], pattern=[[0, 1]], base=0, channel_multiplier=1)
            iota_cf = cst.tile([P, 1], F32)
            nc.vector.tensor_copy(iota_cf[:], iota_ci[:])
            ident_bf = cst.tile([P, P], F32)
            nc.vector.tensor_scalar(out=ident_bf[:], in0=iota_f[:], scalar1=iota_cf[:, :1],
                                    scalar2=None, op0=mybir.AluOpType.is_equal)

            srcidx_t = cst.tile([P, T_pad], I32)
            nc.sync.dma_start(out=srcidx_t[:], in_=srcidx[:])
            srcidx2_t = cst.tile([P, T_pad], I32)
            nc.sync.dma_start(out=srcidx2_t[:], in_=srcidx2[:])
            dstl_t = cst.tile([P, T_pad], F32)
            nc.sync.dma_start(out=dstl_t[:], in_=dstl[:])
            dinv_t = cst.tile([P, TPT], F32)
            nc.sync.dma_start(out=dinv_t[:], in_=dinv_col[:])
            Bp_t = cst.tile([P, TPT * WG], F32)
            nc.sync.dma_start(out=Bp_t[:], in_=Bpool[:])
            W1_t = wp.tile([P, KCH1, HID], F32)
            W1_r = wp.tile([P, KCH1, HID], F32R)
            for k in range(KCH1):
                nc.sync.dma_start(out=W1_t[:, k, :], in_=W1[k * P:(k + 1) * P, :])
                nc.vector.tensor_copy(W1_r[:, k, :], W1_t[:, k, :])
            W2_t = wp.tile([P, HID // P, OUT_C], F32)
            for k in range(HID // P):
                nc.sync.dma_start(out=W2_t[:, k, :], in_=W2[k * P:(k + 1) * P, :])
            b1_t = cst.tile([P, HID], F32)
            b2_t = cst.tile([P, OUT_C], F32)
            if b1_nz:
                nc.sync.dma_start(out=b1_t[:], in_=b1[:])
            if b2_nz:
                nc.sync.dma_start(out=b2_t[:], in_=b2[:])

            # ---- DRAM scratch
            ag_in_s = dram.tile([NPCP, HID], F32)                        # own s~ (node-major)
            s_table = dram.tile([NCORES * NPCP, HID], F32)
            ag_in_z = dram.tile([NPCP, OUT_C], F32)                      # own z~ (node-major)
            z_table = dram.tile([NCORES * NPCP, OUT_C], F32)

            # ================= phase 1: s~ = dinv * (x @ W1) =================
            halves = [(0, 25), (25, 49)]
            for h0, h1_ in halves:
                ncols = (h1_ - h0) * P
                xb = xbp.tile([P, KCH1, 3200], F32, tag="xb")
                for k in range(KCH1):
                    nc.sync.dma_start(out=xb[:, k, :ncols],
                                      in_=xT[k * P:(k + 1) * P, h0 * P:h1_ * P])
                for t in range(h0, h1_):
                    c0 = (t - h0) * P
                    ps = ps_mm.tile([P, HID], F32, tag="agg", space="PSUM")
                    for k in range(KCH1):
                        xr = xrp.tile([P, P], F32R, tag="xr")
                        nc.vector.tensor_copy(xr[:], xb[:, k, c0:c0 + P])
                        nc.tensor.matmul(out=ps[:], lhsT=xr[:], rhs=W1_r[:, k, :],
                                         start=(k == 0), stop=(k == KCH1 - 1))
                    st = hp.tile([P, HID], F32, tag="st")
                    nc.scalar.activation(st[:], ps[:], AF.Copy, scale=dinv_t[:, t:t + 1])
                    nc.scalar.dma_start(out=ag_in_s[t * P:(t + 1) * P, :], in_=st[:])

            # ================= phase 2: AllGather s~ =================
            nc.gpsimd.collective_compute(
                "AllGather", mybir.AluOpType.bypass,
                replica_groups=[list(range(NCORES))],
                ins=[ag_in_s[:]], outs=[s_table[:]],
            )

            # ================= phase 3: layer-1 aggregation =================
            ps_p1 = ps_pool.tile([WG, HID], F32, space="PSUM")
            for t in range(TPT):
                ps = ps_mm.tile([P, HID], F32, tag="agg", space="PSUM")
                for j in range(off[t], off[t] + K[t]):
                    mt = msgp.tile([P, HID], F32, tag="msg")
                    srcbuf = ag_in_s if (j - off[t]) < K_loc[t] else s_table
                    nc.gpsimd.indirect_dma_start(
                        out=mt[:], out_offset=None,
                        in_=srcbuf[:],
                        in_offset=bass.IndirectOffsetOnAxis(ap=srcidx_t[:, j:j + 1], axis=0),
                    )
                    s01 = s01p.tile([P, P], F32, tag="s01")
                    nc.vector.tensor_scalar(out=s01[:], in0=iota_f[:], scalar1=dstl_t[:, j:j + 1],
                                            scalar2=None, op0=mybir.AluOpType.is_equal)
                    nc.tensor.matmul(out=ps[:], lhsT=s01[:], rhs=mt[:],
                                     start=(j == off[t]), stop=False)
                selfm = msgp.tile([P, HID], F32, tag="msg")
                nc.sync.dma_start(out=selfm[:], in_=ag_in_s[t * P:(t + 1) * P, :])
                nc.tensor.matmul(out=ps[:], lhsT=ident_bf[:], rhs=selfm[:], start=False, stop=True)

                h1 = hp.tile([P, HID], F32, tag="h1")
                if b1_nz:
                    tmp = hp.tile([P, HID], F32, tag="tmp1")
                    nc.scalar.activation(tmp[:], ps[:], AF.Copy, scale=dinv_t[:, t:t + 1])
                    nc.vector.tensor_tensor(out=tmp[:], in0=tmp[:], in1=b1_t[:], op=mybir.AluOpType.add)
                    nc.scalar.activation(h1[:], tmp[:], AF.Relu)
                else:
                    nc.scalar.activation(h1[:], ps[:], AF.Relu, scale=dinv_t[:, t:t + 1])
                nc.scalar.dma_start(out=out_nodes[t * P:(t + 1) * P, 0:HID], in_=h1[:])
                nc.tensor.matmul(out=ps_p1[:], lhsT=Bp_t[:, t * WG:(t + 1) * WG], rhs=h1[:],
                                 start=(t == 0), stop=(t == TPT - 1))
                ht = hp.tile([P, HID], F32, tag="ht")
                nc.scalar.activation(ht[:], h1[:], AF.Copy, scale=dinv_t[:, t:t + 1])
                g, sl = t // 4, t % 4
                if sl == 0:
                    hstage = hsp.tile([P, HID // P, 512], F32, tag="hstage")
                for m in range(HID // P):
                    tp = ps_tr.tile([P, P], F32, tag="tr", space="PSUM")
                    nc.tensor.transpose(out=tp[:], in_=ht[:, m * P:(m + 1) * P], identity=ident_bf[:])
                    nc.vector.tensor_copy(hstage[:, m, sl * P:(sl + 1) * P], tp[:])
                if t == min(4 * g + 3, TPT - 1):
                    w = (sl + 1) * P
                    psz = ps_mm.tile([P, 512], F32, tag="zz", space="PSUM")
                    for chunk in range(HID // P):
                        nc.tensor.matmul(out=psz[:, :w], lhsT=W2_t[:, chunk, :],
                                         rhs=hstage[:, chunk, :w],
                                         start=(chunk == 0), stop=(chunk == HID // P - 1))
                    zb = zbp.tile([P, 512], F32, tag="zb")
                    nc.vector.tensor_copy(zb[:, :w], psz[:, :w])
                    for q in range(w // P):
                        tp2 = ps_tr.tile([P, P], F32, tag="tr", space="PSUM")
                        nc.tensor.transpose(out=tp2[:], in_=zb[:, q * P:(q + 1) * P], identity=ident_bf[:])
                        tb2 = zbp.tile([P, P], F32, tag="zt")
                        nc.vector.tensor_copy(tb2[:], tp2[:])
                        row0 = g * 512 + q * P
                        nc.sync.dma_start(out=ag_in_z[row0:row0 + P, :], in_=tb2[:])
                    if (g + 1) * 512 == Z_BOUNDS[1]:
                        lo, hi = Z_BOUNDS[0], Z_BOUNDS[1]
                        nc.gpsimd.collective_compute(
                            "AllGather", mybir.AluOpType.bypass,
                            replica_groups=[list(range(NCORES))],
                            ins=[ag_in_z[lo:hi, :]],
                            outs=[z_table[NCORES * lo:NCORES * hi, :]],
                        )

            pooled1 = hp.tile([WG, HID], F32, tag="pooled1")
            nc.scalar.activation(pooled1[:], ps_p1[:], AF.Copy)
            nc.sync.dma_start(out=out_pooled[:, 0:HID], in_=pooled1[:])

            # ========= phase 5: AllGather z~ (remaining rows) =========
            lo, hi = Z_BOUNDS[1], Z_BOUNDS[2]
            nc.gpsimd.collective_compute(
                "AllGather", mybir.AluOpType.bypass,
                replica_groups=[list(range(NCORES))],
                ins=[ag_in_z[lo:hi, :]],
                outs=[z_table[NCORES * lo:, :]],
            )

            # ================= phase 6: layer-2 aggregation =================
            ps_p2 = ps_pool.tile([WG, OUT_C], F32, space="PSUM")
            for t in range(TPT):
                ps = ps_mm.tile([P, OUT_C], F32, tag="agg", space="PSUM")
                for j in range(off[t], off[t] + K[t]):
                    mt = msgp.tile([P, OUT_C], F32, tag="msg")
                    srcbuf = ag_in_z if (j - off[t]) < K_loc[t] else z_table
                    nc.gpsimd.indirect_dma_start(
                        out=mt[:], out_offset=None,
                        in_=srcbuf[:],
                        in_offset=bass.IndirectOffsetOnAxis(ap=srcidx2_t[:, j:j + 1], axis=0),
                    )
                    s01 = s01p.tile([P, P], F32, tag="s01")
                    nc.vector.tensor_scalar(out=s01[:], in0=iota_f[:], scalar1=dstl_t[:, j:j + 1],
                                            scalar2=None, op0=mybir.AluOpType.is_equal)
                    nc.tensor.matmul(out=ps[:], lhsT=s01[:], rhs=mt[:],
                                     start=(j == off[t]), stop=False)
                selfm = msgp.tile([P, OUT_C], F32, tag="msg")
                nc.sync.dma_start(out=selfm[:], in_=ag_in_z[t * P:(t + 1) * P, :])
                nc.tensor.matmul(out=ps[:], lhsT=ident_bf[:], rhs=selfm[:], start=False, stop=True)

                h2 = hp.tile([P, OUT_C], F32, tag="h2")
                if b2_nz:
                    tmp = hp.tile([P, OUT_C], F32, tag="tmp2")
                    nc.scalar.activation(tmp[:], ps[:], AF.Copy, scale=dinv_t[:, t:t + 1])
                    nc.vector.tensor_tensor(out=tmp[:], in0=tmp[:], in1=b2_t[:], op=mybir.AluOpType.add)
                    nc.scalar.activation(h2[:], tmp[:], AF.Relu)
                else:
                    nc.scalar.activation(h2[:], ps[:], AF.Relu, scale=dinv_t[:, t:t + 1])
                nc.scalar.dma_start(out=out_nodes[t * P:(t + 1) * P, HID:HID + OUT_C], in_=h2[:])
                nc.tensor.matmul(out=ps_p2[:], lhsT=Bp_t[:, t * WG:(t + 1) * WG], rhs=h2[:],
                                 start=(t == 0), stop=(t == TPT - 1))

            pooled2 = hp.tile([WG, OUT_C], F32, tag="pooled2")
            nc.scalar.activation(pooled2[:], ps_p2[:], AF.Copy)
            nc.sync.dma_start(out=out_pooled[:, HID:HID + OUT_C], in_=pooled2[:])

    _split_multi_waits(nc, mybir)
    return nc


_PROGRAM_CACHE = {}


def kernel(x, edge_index, batch, num_graphs, W1, b1, W2, b2):
    from concourse.bass_utils import run_bass_kernel_spmd

    in_maps, meta = _preprocess(x, edge_index, batch, W1, b1, W2, b2)
    cache_key = (tuple(meta["K"]), tuple(meta["K_loc"]), meta["b1_nz"], meta["b2_nz"])
    nc = _PROGRAM_CACHE.get(cache_key)
    if nc is None:
        nc = _build_program(meta)
        _PROGRAM_CACHE[cache_key] = nc

    res = run_bass_kernel_spmd(nc, in_maps, list(range(NCORES))).results

    G = int(num_graphs)
    node_embed = np.concatenate(
        [res[c]["out_nodes"][meta["slot_of"][c]] for c in range(NCORES)], axis=0)
    graph_embed = np.zeros((G + WG, HID + OUT_C), np.float32)
    for c in range(NCORES):
        g0 = int(meta["g_start"][c])
        graph_embed[g0:g0 + WG] += res[c]["out_pooled"]
    graph_embed = graph_embed[:G]
    return graph_embed.astype(np.float32), node_embed.astype(np.float32)
